# revision 37
# baseline (speedup 1.0000x reference)
"""Multi-head attention (B=2, N=2048, C=1024, H=16) on 8 TRN2 NeuronCores.

Sharding: tensor-parallel over heads (2 heads/core) for qkv+attention,
then AllToAll to token-shard the output projection.

Per-core dataflow (bf16 matmuls, fp32 softmax-normalizer chain):
  x^T[b] (host-pretransposed) --matmul w/ W^T shards--> Q^T,K^T [128,N]
  V computed directly in [tok, ch] layout (bf16 full-rate at free dim 128)
  S^T[nk,nq] = K^T.T-slices @ Q^T  (d=64 contraction, 2 heads row-packed:
               bf16 pairs run concurrently in disjoint PE row groups)
  P^T = exp(0.125*S^T)  (ScalarE, scale folded; unsafe softmax - scores ~N(0,1))
  h_aug^T[65,nq] = [V|1].T @ P^T   (row 64 = softmax denominator, free)
  h^T = h_aug^T[0:64] * bcast(1/h_aug^T[64])
        (DVE reciprocal + GPSIMD partition_broadcast + DVE multiply)
  AllToAll per qb-pair (1024-token chunks) -> full-channel h^T slices
  out = h^T.T @ proj_w^T + b  (K=1 bf16 ones matmul adds bias into psum)

Both batches' qkv chains are emitted before either batch's attention so the
PE always has ready filler work and the ScalarE exp stream stays saturated.
"""

import numpy as np
import ml_dtypes

import concourse.mybir as mybir
import concourse.tile as tile
from concourse import bacc
from concourse.bass_utils import run_bass_kernel_spmd

F32 = mybir.dt.float32
F32R = mybir.dt.float32r
BF16 = mybir.dt.bfloat16
EXP = mybir.ActivationFunctionType.Exp

N_CORES = 8
B = 2
C = 1024
H = 16
D = 64
HPC = H // N_CORES          # heads per core
CH = HPC * D                # channels per core (128)
KT_C = C // 128             # contraction tiles (8)


def build_program(N=2048, n_cores=N_CORES, mm_dt=BF16):
    """Build the SPMD Bass program (same on every core)."""
    assert N % 512 == 0
    QB = N // 512            # 512-wide query-column blocks per batch
    NK = N // 128            # 128-row key tiles per batch
    TG = NK // 2             # key-tile pairs (exp granularity 1024)
    scale = float(D) ** -0.5
    # qb-pair groups: each becomes one AllToAll + proj chunk
    QGRP = [list(range(g, min(g + 2, QB))) for g in range(0, QB, 2)]
    if len(QGRP) > 1:
        QGRP_LAST = QGRP[:-1] + [[g] for g in QGRP[-1]]
    else:
        QGRP_LAST = QGRP
    TOKG = [512 * len(g) // n_cores for g in QGRP]   # tokens/core per group

    nc = bacc.Bacc("TRN2", target_bir_lowering=False, debug=False,
                   num_devices=n_cores)

    xT_d = nc.dram_tensor("xT", [B, C, N], mm_dt, kind="ExternalInput")
    wqT_d = nc.dram_tensor("wqT", [128, KT_C * CH], mm_dt, kind="ExternalInput")
    wkT_d = nc.dram_tensor("wkT", [128, KT_C * CH], mm_dt, kind="ExternalInput")
    wvT_d = nc.dram_tensor("wvT", [128, KT_C * CH], mm_dt, kind="ExternalInput")
    pwT_d = nc.dram_tensor("pwT", [128, KT_C * C], mm_dt, kind="ExternalInput")
    onesb_d = nc.dram_tensor("onesb", [1, 128], mm_dt, kind="ExternalInput")
    pbb_d = nc.dram_tensor("pbb", [C], mm_dt, kind="ExternalInput")
    vones_d = nc.dram_tensor("vonesc", [128, NK], mm_dt, kind="ExternalInput")
    # out[b, t0:t0+tokg, :] = this core's output tokens (flat per batch)
    TOKB = N // n_cores
    out_d = nc.dram_tensor("out", [B, TOKB, C], F32, kind="ExternalOutput")

    lp = nc.allow_low_precision("bf16 matmul pipeline")

    with tile.TileContext(nc) as tc:
        with (tc.tile_pool(name="sb", bufs=1) as sb,
              tc.tile_pool(name="ps", bufs=1, space="PSUM") as ps,
              tc.tile_pool(name="dr", bufs=1, space="DRAM") as dr,
              lp):
            # PSUM (8 banks): sst 2x2 + hav0 + hav1 + acc(qkv/v/bcp/warm) + proj

            # ---- constants (host-fed) ----
            onesb = sb.tile([1, 128], mm_dt, tag="onesb", bufs=1)
            nc.sync.dma_start(onesb[:], onesb_d.ap())
            pbb_sb = sb.tile([1, C], mm_dt, tag="pbb", bufs=1)
            nc.sync.dma_start(pbb_sb[:], pbb_d.ap().unsqueeze(0))
            vones = sb.tile([128, NK], mm_dt, tag="vones", bufs=1)
            nc.sync.dma_start(vones[:], vones_d.ap())

            wq, wk, wv, pw = [], [], [], []
            for lst, dram, nm in ((wq, wqT_d, "wq"), (wk, wkT_d, "wk"),
                                  (wv, wvT_d, "wv")):
                    wt = sb.tile([128, KT_C * CH], mm_dt, tag=nm, bufs=1,
                                 name=nm)
                    nc.sync.dma_start(wt[:], dram.ap())
                    for k in range(KT_C):
                        lst.append(wt[:, CH * k:CH * k + CH])

            # warm the PE's HAM clock gate while the x^T DMAs stream
            warm = ps.tile([128, 128], F32, tag="acc", bufs=1, name="warm")
            for i in range(96):
                nc.tensor.matmul(warm[:], wq[i % KT_C], wk[i % KT_C],
                                 start=True, stop=True)

            state = [([], [], []) for _ in range(B)]

            def qkv_gen(b):
                kts, qt, vau = state[b]
                xt = []
                for k in range(KT_C):
                    t = sb.tile([128, N], mm_dt, tag="xt", bufs=9,
                                name=f"xt{b}_{k}")
                    # split per 512-col block: lands on separate DMA queues
                    # and matches per-qb consumption order
                    for c in range(QB):
                        cs_ = slice(512 * c, 512 * c + 512)
                        nc.sync.dma_start(
                            t[:, cs_], xT_d.ap()[b, 128 * k:128 * k + 128, cs_])
                    xt.append(t)
                for h in range(2):
                    t = sb.tile([128, 65 * NK], mm_dt, tag=f"vau{h}", bufs=2,
                                name=f"vau{b}_{h}")
                    nc.sync.dma_start(t[:, 64::65], vones_d.ap())
                    vau.append(t)

                def k_chain(qb):
                    cs = slice(512 * qb, 512 * qb + 512)
                    acc = ps.tile([128, 512], F32, tag="acc", bufs=1,
                                  name=f"kacc{b}_{qb}")
                    for k in range(KT_C):
                        nc.tensor.matmul(acc[:], wk[k], xt[k][:, cs],
                                         start=(k == 0), stop=(k == KT_C - 1))
                        yield
                    ktile = sb.tile([128, 512], mm_dt, tag="kt", bufs=9,
                                    name=f"kt{b}_{qb}")
                    nc.vector.tensor_copy(ktile[:], acc[:])
                    kts.append(ktile)

                def q_chain(qb):
                    cs = slice(512 * qb, 512 * qb + 512)
                    acc = ps.tile([128, 512], F32, tag="acc", bufs=1,
                                  name=f"qacc{b}_{qb}")
                    for k in range(KT_C):
                        nc.tensor.matmul(acc[:], wq[k], xt[k][:, cs],
                                         start=(k == 0), stop=(k == KT_C - 1))
                        yield
                    qtile = sb.tile([128, 512], mm_dt, tag="qt", bufs=9,
                                    name=f"qt{b}_{qb}")
                    nc.vector.tensor_copy(qtile[:], acc[:])
                    qt.append(qtile)

                for qb in range(QB):
                    yield from k_chain(qb)
                yield from q_chain(0)
                # V directly in [tok, ch] layout (bf16: full rate at N=128)
                for tt in range(NK):
                    ts_ = slice(128 * tt, 128 * tt + 128)
                    acc = ps.tile([128, 128], F32, tag="acc", bufs=1,
                                  name=f"vacc{b}_{tt}")
                    for k in range(KT_C):
                        nc.tensor.matmul(acc[:], xt[k][:, ts_], wv[k],
                                         start=(k == 0), stop=(k == KT_C - 1))
                        yield
                    nc.vector.tensor_copy(
                        vau[0][:, 65 * tt:65 * tt + 64], acc[:, 0:64])
                    nc.vector.tensor_copy(
                        vau[1][:, 65 * tt:65 * tt + 64], acc[:, 64:128])
                for qb in range(1, QB):
                    yield from q_chain(qb)

            def pull(g, n):
                if g is None:
                    return
                for _ in range(n):
                    if next(g, "done") == "done":
                        return

            gens = [qkv_gen(b) for b in range(B)]
            pull(gens[0], 10 ** 9)  # first batch's qkv emitted up front

            for b in range(B):
                kts, qt, vau = state[b]
                filler = gens[b + 1] if b + 1 < B else None
                # ---- attention, grouped by qb-pairs for chunked AllToAll ----
                grps = QGRP_LAST if b == B - 1 else QGRP
                tok0 = 0
                for gi, grp in enumerate(grps):
                    tokg = 512 * len(grp) // n_cores
                    a2a_in = dr.tile([n_cores, CH, tokg], mm_dt,
                                     tag="a2ain", bufs=2, name=f"a2ain{b}_{gi}")
                    hcps = []
                    for p, qb in enumerate(grp):
                        hav = [ps.tile([65, 512], F32, tag=f"hav{h}", bufs=1,
                                       name=f"hav{b}_{qb}_{h}")
                               for h in range(2)]
                        for t_idx in range(NK):
                            # one tile per key-tile: [h0 | h1] halves, so the
                            # two S^T matmuls alternate PE row groups (paired)
                            sst = ps.tile([128, 1024], F32, tag="sst", bufs=2,
                                          name=f"sst{b}_{qb}_{t_idx}")
                            pt = sb.tile([128, 1024], mm_dt, tag="pt", bufs=6,
                                         name=f"pt{b}_{qb}_{t_idx}")
                            kb, ko = t_idx // 4, t_idx % 4
                            ks = slice(128 * ko, 128 * ko + 128)
                            for h in range(2):
                                hs = slice(64 * h, 64 * h + 64)
                                nc.tensor.matmul(
                                    sst[:, 512 * h:512 * h + 512],
                                    kts[kb][hs, ks], qt[qb][hs, :],
                                    start=True, stop=True)
                            nc.scalar.activation(pt[:], sst[:], EXP,
                                                 scale=scale)
                            for h in range(2):
                                nc.tensor.matmul(
                                    hav[h][:],
                                    vau[h][:, 65 * t_idx:65 * t_idx + 65],
                                    pt[:, 512 * h:512 * h + 512],
                                    start=(t_idx == 0),
                                    stop=(t_idx == NK - 1))
                            # interleave a few of the next batch's qkv matmuls
                            pull(filler, 3)
                        # copy h_aug out of PSUM fast so hav slots recycle
                        hcp = [sb.tile([65, 512], F32, tag=f"hcp{h}", bufs=2,
                                       name=f"hcp{b}_{qb}_{h}")
                               for h in range(2)]
                        for h in range(2):
                            nc.vector.tensor_copy(hcp[h][:], hav[h][:])
                        hcps.append(hcp)
                    for p, qb in enumerate(grp):
                        hcp = hcps[p]
                        ht = sb.tile([128, 512], mm_dt, tag="ht", bufs=2,
                                     name=f"ht{b}_{qb}")
                        for h in range(2):
                            nrr = sb.tile([1, 512], F32, tag=f"nrr{h}",
                                          bufs=2, name=f"nrr{b}_{qb}_{h}")
                            nc.vector.reciprocal(nrr[:], hcp[h][64:65, :])
                            bcs = sb.tile([64, 512], F32, tag=f"bcs{h}",
                                          bufs=2, name=f"bcs{b}_{qb}_{h}")
                            nc.gpsimd.partition_broadcast(bcs[:], nrr[:])
                            nc.vector.tensor_mul(ht[64 * h:64 * h + 64, :],
                                                 hcp[h][0:64, :], bcs[:])
                        # scatter this qb's tokens into the group A2A buffer
                        npr = 512 // tokg
                        for s in range(npr):
                            j = p * npr + s
                            nc.sync.dma_start(
                                a2a_in[j][:, 0:tokg],
                                ht[:, tokg * s:tokg * s + tokg])

                    # ---- AllToAll: head-sharded -> token-sharded ----
                    a2a_out = dr.tile([n_cores, CH, tokg], mm_dt,
                                      tag="a2aout", bufs=2,
                                      name=f"a2aout{b}_{gi}")
                    nc.gpsimd.collective_compute(
                        "AllToAll", mybir.AluOpType.bypass,
                        replica_groups=[list(range(n_cores))],
                        ins=[a2a_in.opt()], outs=[a2a_out.opt()])

                    # ---- output projection for this group's tokens ----
                    if not pw:
                        pwt = sb.tile([128, KT_C * C], mm_dt, tag="pw",
                                      bufs=1, name="pw")
                        nc.sync.dma_start(pwt[:], pwT_d.ap())
                        for k in range(KT_C):
                            pw.append(pwt[:, C * k:C * k + C])
                    pl = []
                    for k in range(KT_C):
                        t = sb.tile([128, tokg], mm_dt, tag=f"pl{k}", bufs=2,
                                    name=f"pl{b}_{gi}_{k}")
                        nc.sync.dma_start(t[:], a2a_out[k])
                        pl.append(t)
                    for oh in range(2):
                        os_ = slice(512 * oh, 512 * oh + 512)
                        acc = ps.tile([128, 512], F32, tag="proj", bufs=1,
                                      name=f"pacc{b}_{gi}_{oh}")
                        for k in range(KT_C):
                            nc.tensor.matmul(
                                acc[0:tokg, :], pl[k][:], pw[k][:, os_],
                                start=(k == 0), stop=False)
                        nc.tensor.matmul(acc[0:tokg, :], onesb[0:1, 0:tokg],
                                         pbb_sb[0:1, os_],
                                         start=False, stop=True)
                        osb = sb.tile([128, 512], F32, tag="osb", bufs=2,
                                      name=f"osb{b}_{gi}_{oh}")
                        nc.vector.tensor_copy(osb[0:tokg, :], acc[0:tokg, :])
                        nc.sync.dma_start(
                            out_d.ap()[b, tok0:tok0 + tokg, os_],
                            osb[0:tokg, :])
                    tok0 += tokg
                pull(filler, 10 ** 9)

    nc.compile()
    return nc


def shard_inputs(x, qkv_w, proj_w, proj_b, n_cores=N_CORES, mm_dt=BF16):
    """Host-side sharding: pre-transpose activations/weights, slice heads."""
    npdt = ml_dtypes.bfloat16 if mm_dt == BF16 else np.float32
    xT = np.ascontiguousarray(
        np.transpose(np.asarray(x), (0, 2, 1))).astype(npdt)
    qkv_w = np.asarray(qkv_w)

    def pack(wT):  # [C, cols] -> [128, (C//128)*cols], k-tiles side by side
        cdim, cols = wT.shape
        return np.ascontiguousarray(
            wT.reshape(cdim // 128, 128, cols).transpose(1, 0, 2)
            .reshape(128, -1)).astype(npdt)

    pwT = pack(np.asarray(proj_w).T)
    pb = np.ascontiguousarray(np.asarray(proj_b)).astype(np.float32)
    nk = x.shape[1] // 128
    vonesc = np.ones((128, nk), dtype=npdt)
    in_maps = []
    for i in range(n_cores):
        cs = slice(CH * i, CH * i + CH)
        in_maps.append({
            "xT": xT,
            "wqT": pack(qkv_w[cs, :].T),
            "wkT": pack(qkv_w[C:][cs, :].T),
            "wvT": pack(qkv_w[2 * C:][cs, :].T),
            "pwT": pwT,
            "onesb": np.ones((1, 128), dtype=npdt),
            "pbb": pb.astype(npdt),
            "vonesc": vonesc,
        })
    return in_maps


def assemble_output(res, N, n_cores=N_CORES):
    QB = N // 512
    QGRP = [list(range(g, min(g + 2, QB))) for g in range(0, QB, 2)]
    if len(QGRP) > 1:
        QGRP_LAST = QGRP[:-1] + [[g] for g in QGRP[-1]]
    else:
        QGRP_LAST = QGRP
    out = np.empty((B, N, C), dtype=np.float32)
    for i in range(n_cores):
        o = res.results[i]["out"]  # [B, TOKB, C]
        for b in range(B):
            grps = QGRP_LAST if b == B - 1 else QGRP
            tok0 = 0
            base = 0
            for grp in grps:
                tokg = 512 * len(grp) // n_cores
                lo = base + tokg * i
                out[b, lo:lo + tokg, :] = o[b, tok0:tok0 + tokg]
                tok0 += tokg
                base += 512 * len(grp)
    return out


_NC_CACHE = {}


def _get_program(N, mm_dt=BF16):
    key = (N, str(mm_dt))
    if key not in _NC_CACHE:
        _NC_CACHE[key] = build_program(N=N, mm_dt=mm_dt)
    return _NC_CACHE[key]


def kernel(x, qkv_w, proj_w, proj_b):
    x = np.asarray(x)
    Bx, N, Cx = x.shape
    assert (Bx, Cx) == (B, C), (Bx, Cx)
    nc = _get_program(N)
    in_maps = shard_inputs(x, qkv_w, proj_w, proj_b)
    res = run_bass_kernel_spmd(nc, in_maps, list(range(N_CORES)))
    return assemble_output(res, N)


# revision 38
# speedup vs baseline: 1.0061x; 1.0061x over previous
"""Multi-head attention (B=2, N=2048, C=1024, H=16) on 8 TRN2 NeuronCores.

Sharding: tensor-parallel over heads (2 heads/core) for qkv+attention,
then AllToAll to token-shard the output projection.

Per-core dataflow (bf16 matmuls, fp32 softmax-normalizer chain):
  x^T[b] (host-pretransposed) --matmul w/ W^T shards--> Q^T,K^T [128,N]
  V computed directly in [tok, ch] layout (bf16 full-rate at free dim 128)
  S^T[nk,nq] = K^T.T-slices @ Q^T  (d=64 contraction, 2 heads row-packed:
               bf16 pairs run concurrently in disjoint PE row groups)
  P^T = exp(0.125*S^T)  (ScalarE, scale folded; unsafe softmax - scores ~N(0,1))
  h_aug^T[65,nq] = [V|1].T @ P^T   (row 64 = softmax denominator, free)
  h^T = h_aug^T[0:64] * bcast(1/h_aug^T[64])
        (DVE reciprocal + GPSIMD partition_broadcast + DVE multiply)
  AllToAll per qb-pair (1024-token chunks) -> full-channel h^T slices
  out = h^T.T @ proj_w^T + b  (K=1 bf16 ones matmul adds bias into psum)

Both batches' qkv chains are emitted before either batch's attention so the
PE always has ready filler work and the ScalarE exp stream stays saturated.
"""

import numpy as np
import ml_dtypes

import concourse.mybir as mybir
import concourse.tile as tile
from concourse import bacc
from concourse.bass_utils import run_bass_kernel_spmd

F32 = mybir.dt.float32
F32R = mybir.dt.float32r
BF16 = mybir.dt.bfloat16
EXP = mybir.ActivationFunctionType.Exp

N_CORES = 8
B = 2
C = 1024
H = 16
D = 64
HPC = H // N_CORES          # heads per core
CH = HPC * D                # channels per core (128)
KT_C = C // 128             # contraction tiles (8)


def build_program(N=2048, n_cores=N_CORES, mm_dt=BF16):
    """Build the SPMD Bass program (same on every core)."""
    assert N % 512 == 0
    QB = N // 512            # 512-wide query-column blocks per batch
    NK = N // 128            # 128-row key tiles per batch
    TG = NK // 2             # key-tile pairs (exp granularity 1024)
    scale = float(D) ** -0.5
    # qb-pair groups: each becomes one AllToAll + proj chunk
    QGRP = [list(range(g, min(g + 2, QB))) for g in range(0, QB, 2)]
    if len(QGRP) > 1:
        QGRP_LAST = QGRP[:-1] + [[g] for g in QGRP[-1]]
    else:
        QGRP_LAST = QGRP
    TOKG = [512 * len(g) // n_cores for g in QGRP]   # tokens/core per group

    nc = bacc.Bacc("TRN2", target_bir_lowering=False, debug=False,
                   num_devices=n_cores)

    xT_d = nc.dram_tensor("xT", [B, C, N], mm_dt, kind="ExternalInput")
    wqT_d = nc.dram_tensor("wqT", [128, KT_C * CH], mm_dt, kind="ExternalInput")
    wkT_d = nc.dram_tensor("wkT", [128, KT_C * CH], mm_dt, kind="ExternalInput")
    wvT_d = nc.dram_tensor("wvT", [128, KT_C * CH], mm_dt, kind="ExternalInput")
    pwT_d = nc.dram_tensor("pwT", [128, KT_C * C], mm_dt, kind="ExternalInput")
    onesb_d = nc.dram_tensor("onesb", [1, 128], mm_dt, kind="ExternalInput")
    pbb_d = nc.dram_tensor("pbb", [C], mm_dt, kind="ExternalInput")
    vones_d = nc.dram_tensor("vonesc", [128, NK], mm_dt, kind="ExternalInput")
    # out[b, t0:t0+tokg, :] = this core's output tokens (flat per batch)
    TOKB = N // n_cores
    out_d = nc.dram_tensor("out", [B, TOKB, C], F32, kind="ExternalOutput")

    lp = nc.allow_low_precision("bf16 matmul pipeline")

    with tile.TileContext(nc) as tc:
        with (tc.tile_pool(name="sb", bufs=1) as sb,
              tc.tile_pool(name="ps", bufs=1, space="PSUM") as ps,
              tc.tile_pool(name="dr", bufs=1, space="DRAM") as dr,
              lp):
            # PSUM (8 banks): sst 2x2 + hav0 + hav1 + acc(qkv/v/bcp/warm) + proj

            # ---- constants (host-fed) ----
            onesb = sb.tile([1, 128], mm_dt, tag="onesb", bufs=1)
            nc.sync.dma_start(onesb[:], onesb_d.ap())
            pbb_sb = sb.tile([1, C], mm_dt, tag="pbb", bufs=1)
            nc.sync.dma_start(pbb_sb[:], pbb_d.ap().unsqueeze(0))
            vones = sb.tile([128, NK], mm_dt, tag="vones", bufs=1)
            nc.sync.dma_start(vones[:], vones_d.ap())

            wq, wk, wv, pw = [], [], [], []
            for lst, dram, nm in ((wq, wqT_d, "wq"), (wk, wkT_d, "wk"),
                                  (wv, wvT_d, "wv")):
                    wt = sb.tile([128, KT_C * CH], mm_dt, tag=nm, bufs=1,
                                 name=nm)
                    nc.sync.dma_start(wt[:], dram.ap())
                    for k in range(KT_C):
                        lst.append(wt[:, CH * k:CH * k + CH])

            # warm the PE's HAM clock gate while the x^T DMAs stream
            warm = ps.tile([128, 128], F32, tag="acc", bufs=1, name="warm")
            for i in range(160):
                nc.tensor.matmul(warm[:], wq[i % KT_C], wk[i % KT_C],
                                 start=True, stop=True)

            state = [([], [], []) for _ in range(B)]

            def qkv_gen(b):
                kts, qt, vau = state[b]
                xt = []
                for k in range(KT_C):
                    t = sb.tile([128, N], mm_dt, tag="xt", bufs=9,
                                name=f"xt{b}_{k}")
                    # split by partition halves: two DMA queues per tile,
                    # full-width 4KB rows preserved
                    for ph in range(2):
                        рs = slice(64 * ph, 64 * ph + 64)
                        nc.sync.dma_start(
                            t[рs, :],
                            xT_d.ap()[b, 128 * k + 64 * ph:128 * k + 64 * ph + 64, :])
                    xt.append(t)
                for h in range(2):
                    t = sb.tile([128, 65 * NK], mm_dt, tag=f"vau{h}", bufs=2,
                                name=f"vau{b}_{h}")
                    nc.sync.dma_start(t[:, 64::65], vones_d.ap())
                    vau.append(t)

                def k_chain(qb):
                    cs = slice(512 * qb, 512 * qb + 512)
                    acc = ps.tile([128, 512], F32, tag="acc", bufs=1,
                                  name=f"kacc{b}_{qb}")
                    for k in range(KT_C):
                        nc.tensor.matmul(acc[:], wk[k], xt[k][:, cs],
                                         start=(k == 0), stop=(k == KT_C - 1))
                        yield
                    ktile = sb.tile([128, 512], mm_dt, tag="kt", bufs=9,
                                    name=f"kt{b}_{qb}")
                    nc.vector.tensor_copy(ktile[:], acc[:])
                    kts.append(ktile)

                def q_chain(qb):
                    cs = slice(512 * qb, 512 * qb + 512)
                    acc = ps.tile([128, 512], F32, tag="acc", bufs=1,
                                  name=f"qacc{b}_{qb}")
                    for k in range(KT_C):
                        nc.tensor.matmul(acc[:], wq[k], xt[k][:, cs],
                                         start=(k == 0), stop=(k == KT_C - 1))
                        yield
                    qtile = sb.tile([128, 512], mm_dt, tag="qt", bufs=9,
                                    name=f"qt{b}_{qb}")
                    nc.vector.tensor_copy(qtile[:], acc[:])
                    qt.append(qtile)

                for qb in range(QB):
                    yield from k_chain(qb)
                yield from q_chain(0)
                # V directly in [tok, ch] layout (bf16: full rate at N=128)
                for tt in range(NK):
                    ts_ = slice(128 * tt, 128 * tt + 128)
                    acc = ps.tile([128, 128], F32, tag="acc", bufs=1,
                                  name=f"vacc{b}_{tt}")
                    for k in range(KT_C):
                        nc.tensor.matmul(acc[:], xt[k][:, ts_], wv[k],
                                         start=(k == 0), stop=(k == KT_C - 1))
                        yield
                    nc.vector.tensor_copy(
                        vau[0][:, 65 * tt:65 * tt + 64], acc[:, 0:64])
                    nc.vector.tensor_copy(
                        vau[1][:, 65 * tt:65 * tt + 64], acc[:, 64:128])
                for qb in range(1, QB):
                    yield from q_chain(qb)

            def pull(g, n):
                if g is None:
                    return
                for _ in range(n):
                    if next(g, "done") == "done":
                        return

            gens = [qkv_gen(b) for b in range(B)]
            pull(gens[0], 10 ** 9)  # first batch's qkv emitted up front

            for b in range(B):
                kts, qt, vau = state[b]
                filler = gens[b + 1] if b + 1 < B else None
                # ---- attention, grouped by qb-pairs for chunked AllToAll ----
                grps = QGRP_LAST if b == B - 1 else QGRP
                tok0 = 0
                for gi, grp in enumerate(grps):
                    tokg = 512 * len(grp) // n_cores
                    a2a_in = dr.tile([n_cores, CH, tokg], mm_dt,
                                     tag="a2ain", bufs=2, name=f"a2ain{b}_{gi}")
                    hcps = []
                    for p, qb in enumerate(grp):
                        hav = [ps.tile([65, 512], F32, tag=f"hav{h}", bufs=1,
                                       name=f"hav{b}_{qb}_{h}")
                               for h in range(2)]
                        for t_idx in range(NK):
                            # one tile per key-tile: [h0 | h1] halves, so the
                            # two S^T matmuls alternate PE row groups (paired)
                            sst = ps.tile([128, 1024], F32, tag="sst", bufs=2,
                                          name=f"sst{b}_{qb}_{t_idx}")
                            pt = sb.tile([128, 1024], mm_dt, tag="pt", bufs=6,
                                         name=f"pt{b}_{qb}_{t_idx}")
                            kb, ko = t_idx // 4, t_idx % 4
                            ks = slice(128 * ko, 128 * ko + 128)
                            for h in range(2):
                                hs = slice(64 * h, 64 * h + 64)
                                nc.tensor.matmul(
                                    sst[:, 512 * h:512 * h + 512],
                                    kts[kb][hs, ks], qt[qb][hs, :],
                                    start=True, stop=True)
                            nc.scalar.activation(pt[:], sst[:], EXP,
                                                 scale=scale)
                            for h in range(2):
                                nc.tensor.matmul(
                                    hav[h][:],
                                    vau[h][:, 65 * t_idx:65 * t_idx + 65],
                                    pt[:, 512 * h:512 * h + 512],
                                    start=(t_idx == 0),
                                    stop=(t_idx == NK - 1))
                            # interleave a few of the next batch's qkv matmuls
                            pull(filler, 3)
                        # copy h_aug out of PSUM fast so hav slots recycle
                        hcp = [sb.tile([65, 512], F32, tag=f"hcp{h}", bufs=2,
                                       name=f"hcp{b}_{qb}_{h}")
                               for h in range(2)]
                        for h in range(2):
                            nc.vector.tensor_copy(hcp[h][:], hav[h][:])
                        hcps.append(hcp)
                    for p, qb in enumerate(grp):
                        hcp = hcps[p]
                        ht = sb.tile([128, 512], mm_dt, tag="ht", bufs=2,
                                     name=f"ht{b}_{qb}")
                        for h in range(2):
                            nrr = sb.tile([1, 512], F32, tag=f"nrr{h}",
                                          bufs=2, name=f"nrr{b}_{qb}_{h}")
                            nc.vector.reciprocal(nrr[:], hcp[h][64:65, :])
                            bcs = sb.tile([64, 512], F32, tag=f"bcs{h}",
                                          bufs=2, name=f"bcs{b}_{qb}_{h}")
                            nc.gpsimd.partition_broadcast(bcs[:], nrr[:])
                            nc.vector.tensor_mul(ht[64 * h:64 * h + 64, :],
                                                 hcp[h][0:64, :], bcs[:])
                        # scatter this qb's tokens into the group A2A buffer
                        npr = 512 // tokg
                        for s in range(npr):
                            j = p * npr + s
                            nc.sync.dma_start(
                                a2a_in[j][:, 0:tokg],
                                ht[:, tokg * s:tokg * s + tokg])

                    # ---- AllToAll: head-sharded -> token-sharded ----
                    a2a_out = dr.tile([n_cores, CH, tokg], mm_dt,
                                      tag="a2aout", bufs=2,
                                      name=f"a2aout{b}_{gi}")
                    nc.gpsimd.collective_compute(
                        "AllToAll", mybir.AluOpType.bypass,
                        replica_groups=[list(range(n_cores))],
                        ins=[a2a_in.opt()], outs=[a2a_out.opt()])

                    # ---- output projection for this group's tokens ----
                    if not pw:
                        pwt = sb.tile([128, KT_C * C], mm_dt, tag="pw",
                                      bufs=1, name="pw")
                        nc.sync.dma_start(pwt[:], pwT_d.ap())
                        for k in range(KT_C):
                            pw.append(pwt[:, C * k:C * k + C])
                    pl = []
                    for k in range(KT_C):
                        t = sb.tile([128, tokg], mm_dt, tag=f"pl{k}", bufs=2,
                                    name=f"pl{b}_{gi}_{k}")
                        nc.sync.dma_start(t[:], a2a_out[k])
                        pl.append(t)
                    for oh in range(2):
                        os_ = slice(512 * oh, 512 * oh + 512)
                        acc = ps.tile([128, 512], F32, tag="proj", bufs=1,
                                      name=f"pacc{b}_{gi}_{oh}")
                        for k in range(KT_C):
                            nc.tensor.matmul(
                                acc[0:tokg, :], pl[k][:], pw[k][:, os_],
                                start=(k == 0), stop=False)
                        nc.tensor.matmul(acc[0:tokg, :], onesb[0:1, 0:tokg],
                                         pbb_sb[0:1, os_],
                                         start=False, stop=True)
                        osb = sb.tile([128, 512], F32, tag="osb", bufs=2,
                                      name=f"osb{b}_{gi}_{oh}")
                        nc.vector.tensor_copy(osb[0:tokg, :], acc[0:tokg, :])
                        nc.sync.dma_start(
                            out_d.ap()[b, tok0:tok0 + tokg, os_],
                            osb[0:tokg, :])
                    tok0 += tokg
                pull(filler, 10 ** 9)

    nc.compile()
    return nc


def shard_inputs(x, qkv_w, proj_w, proj_b, n_cores=N_CORES, mm_dt=BF16):
    """Host-side sharding: pre-transpose activations/weights, slice heads."""
    npdt = ml_dtypes.bfloat16 if mm_dt == BF16 else np.float32
    xT = np.ascontiguousarray(
        np.transpose(np.asarray(x), (0, 2, 1))).astype(npdt)
    qkv_w = np.asarray(qkv_w)

    def pack(wT):  # [C, cols] -> [128, (C//128)*cols], k-tiles side by side
        cdim, cols = wT.shape
        return np.ascontiguousarray(
            wT.reshape(cdim // 128, 128, cols).transpose(1, 0, 2)
            .reshape(128, -1)).astype(npdt)

    pwT = pack(np.asarray(proj_w).T)
    pb = np.ascontiguousarray(np.asarray(proj_b)).astype(np.float32)
    nk = x.shape[1] // 128
    vonesc = np.ones((128, nk), dtype=npdt)
    in_maps = []
    for i in range(n_cores):
        cs = slice(CH * i, CH * i + CH)
        in_maps.append({
            "xT": xT,
            "wqT": pack(qkv_w[cs, :].T),
            "wkT": pack(qkv_w[C:][cs, :].T),
            "wvT": pack(qkv_w[2 * C:][cs, :].T),
            "pwT": pwT,
            "onesb": np.ones((1, 128), dtype=npdt),
            "pbb": pb.astype(npdt),
            "vonesc": vonesc,
        })
    return in_maps


def assemble_output(res, N, n_cores=N_CORES):
    QB = N // 512
    QGRP = [list(range(g, min(g + 2, QB))) for g in range(0, QB, 2)]
    if len(QGRP) > 1:
        QGRP_LAST = QGRP[:-1] + [[g] for g in QGRP[-1]]
    else:
        QGRP_LAST = QGRP
    out = np.empty((B, N, C), dtype=np.float32)
    for i in range(n_cores):
        o = res.results[i]["out"]  # [B, TOKB, C]
        for b in range(B):
            grps = QGRP_LAST if b == B - 1 else QGRP
            tok0 = 0
            base = 0
            for grp in grps:
                tokg = 512 * len(grp) // n_cores
                lo = base + tokg * i
                out[b, lo:lo + tokg, :] = o[b, tok0:tok0 + tokg]
                tok0 += tokg
                base += 512 * len(grp)
    return out


_NC_CACHE = {}


def _get_program(N, mm_dt=BF16):
    key = (N, str(mm_dt))
    if key not in _NC_CACHE:
        _NC_CACHE[key] = build_program(N=N, mm_dt=mm_dt)
    return _NC_CACHE[key]


def kernel(x, qkv_w, proj_w, proj_b):
    x = np.asarray(x)
    Bx, N, Cx = x.shape
    assert (Bx, Cx) == (B, C), (Bx, Cx)
    nc = _get_program(N)
    in_maps = shard_inputs(x, qkv_w, proj_w, proj_b)
    res = run_bass_kernel_spmd(nc, in_maps, list(range(N_CORES)))
    return assemble_output(res, N)


# revision 39
# speedup vs baseline: 1.0366x; 1.0303x over previous
"""Multi-head attention (B=2, N=2048, C=1024, H=16) on 8 TRN2 NeuronCores.

Sharding: tensor-parallel over heads (2 heads/core) for qkv+attention,
then AllToAll to token-shard the output projection.

Per-core dataflow (bf16 matmuls, fp32 softmax-normalizer chain):
  x^T[b] (host-pretransposed) --matmul w/ W^T shards--> Q^T,K^T [128,N]
  V computed directly in [tok, ch] layout (bf16 full-rate at free dim 128)
  S^T[nk,nq] = K^T.T-slices @ Q^T  (d=64 contraction, 2 heads row-packed:
               bf16 pairs run concurrently in disjoint PE row groups)
  P^T = exp(0.125*S^T)  (ScalarE, scale folded; unsafe softmax - scores ~N(0,1))
  h_aug^T[65,nq] = [V|1].T @ P^T   (row 64 = softmax denominator, free)
  h^T = h_aug^T[0:64] * bcast(1/h_aug^T[64])
        (DVE reciprocal + GPSIMD partition_broadcast + DVE multiply)
  AllToAll per qb-pair (1024-token chunks) -> full-channel h^T slices
  out = h^T.T @ proj_w^T + b  (K=1 bf16 ones matmul adds bias into psum)

Both batches' qkv chains are emitted before either batch's attention so the
PE always has ready filler work and the ScalarE exp stream stays saturated.
"""

import numpy as np
import ml_dtypes

import concourse.mybir as mybir
import concourse.tile as tile
from concourse import bacc
from concourse.bass_utils import run_bass_kernel_spmd

F32 = mybir.dt.float32
F32R = mybir.dt.float32r
BF16 = mybir.dt.bfloat16
EXP = mybir.ActivationFunctionType.Exp

N_CORES = 8
B = 2
C = 1024
H = 16
D = 64
HPC = H // N_CORES          # heads per core
CH = HPC * D                # channels per core (128)
KT_C = C // 128             # contraction tiles (8)


def build_program(N=2048, n_cores=N_CORES, mm_dt=BF16):
    """Build the SPMD Bass program (same on every core)."""
    assert N % 512 == 0
    QB = N // 512            # 512-wide query-column blocks per batch
    NK = N // 128            # 128-row key tiles per batch
    TG = NK // 2             # key-tile pairs (exp granularity 1024)
    scale = float(D) ** -0.5
    # qb-pair groups: each becomes one AllToAll + proj chunk
    QGRP = [list(range(g, min(g + 2, QB))) for g in range(0, QB, 2)]
    if len(QGRP) > 1:
        QGRP_LAST = QGRP[:-1] + [[g] for g in QGRP[-1]]
    else:
        QGRP_LAST = QGRP
    TOKG = [512 * len(g) // n_cores for g in QGRP]   # tokens/core per group

    nc = bacc.Bacc("TRN2", target_bir_lowering=False, debug=False,
                   num_devices=n_cores)

    xT_d = nc.dram_tensor("xT", [B, C, N], mm_dt, kind="ExternalInput")
    wqT_d = nc.dram_tensor("wqT", [128, KT_C * CH], mm_dt, kind="ExternalInput")
    wkT_d = nc.dram_tensor("wkT", [128, KT_C * CH], mm_dt, kind="ExternalInput")
    wvT_d = nc.dram_tensor("wvT", [128, KT_C * CH], mm_dt, kind="ExternalInput")
    pwT_d = nc.dram_tensor("pwT", [128, KT_C * C], mm_dt, kind="ExternalInput")
    onesb_d = nc.dram_tensor("onesb", [1, 128], mm_dt, kind="ExternalInput")
    pbb_d = nc.dram_tensor("pbb", [C], mm_dt, kind="ExternalInput")
    vones_d = nc.dram_tensor("vonesc", [128, NK], mm_dt, kind="ExternalInput")
    # out[b, t0:t0+tokg, :] = this core's output tokens (flat per batch)
    TOKB = N // n_cores
    out_d = nc.dram_tensor("out", [B, TOKB, C], F32, kind="ExternalOutput")

    lp = nc.allow_low_precision("bf16 matmul pipeline")

    with tile.TileContext(nc) as tc:
        with (tc.tile_pool(name="sb", bufs=1) as sb,
              tc.tile_pool(name="ps", bufs=1, space="PSUM") as ps,
              tc.tile_pool(name="dr", bufs=1, space="DRAM") as dr,
              lp):
            # PSUM (8 banks): sst 2x2 + hav0 + hav1 + acc(qkv/v/bcp/warm) + proj

            # ---- constants (host-fed) ----
            onesb = sb.tile([1, 128], mm_dt, tag="onesb", bufs=1)
            nc.sync.dma_start(onesb[:], onesb_d.ap())
            pbb_sb = sb.tile([1, C], mm_dt, tag="pbb", bufs=1)
            nc.sync.dma_start(pbb_sb[:], pbb_d.ap().unsqueeze(0))
            vones = sb.tile([128, NK], mm_dt, tag="vones", bufs=1)
            nc.sync.dma_start(vones[:], vones_d.ap())

            wq, wk, wv, pw = [], [], [], []
            for lst, dram, nm in ((wq, wqT_d, "wq"), (wk, wkT_d, "wk"),
                                  (wv, wvT_d, "wv")):
                    wt = sb.tile([128, KT_C * CH], mm_dt, tag=nm, bufs=1,
                                 name=nm)
                    nc.sync.dma_start(wt[:], dram.ap())
                    for k in range(KT_C):
                        lst.append(wt[:, CH * k:CH * k + CH])

            # warm the PE's HAM clock gate while the x^T DMAs stream
            warm = ps.tile([128, 128], F32, tag="acc", bufs=1, name="warm")
            for i in range(160):
                nc.tensor.matmul(warm[:], wq[i % KT_C], wk[i % KT_C],
                                 start=True, stop=True)

            state = [([], [], []) for _ in range(B)]

            def qkv_gen(b):
                kts, qt, vau = state[b]
                xt = []
                for k in range(KT_C):
                    t = sb.tile([128, N], mm_dt, tag="xt", bufs=9,
                                name=f"xt{b}_{k}")
                    nc.sync.dma_start(
                        t[:], xT_d.ap()[b, 128 * k:128 * k + 128, :])
                    xt.append(t)
                for h in range(2):
                    t = sb.tile([128, 65 * NK], mm_dt, tag=f"vau{h}", bufs=2,
                                name=f"vau{b}_{h}")
                    nc.sync.dma_start(t[:, 64::65], vones_d.ap())
                    vau.append(t)

                def k_chain(qb):
                    cs = slice(512 * qb, 512 * qb + 512)
                    acc = ps.tile([128, 512], F32, tag="acc", bufs=1,
                                  name=f"kacc{b}_{qb}")
                    for k in range(KT_C):
                        nc.tensor.matmul(acc[:], wk[k], xt[k][:, cs],
                                         start=(k == 0), stop=(k == KT_C - 1))
                        yield
                    ktile = sb.tile([128, 512], mm_dt, tag="kt", bufs=9,
                                    name=f"kt{b}_{qb}")
                    nc.vector.tensor_copy(ktile[:], acc[:])
                    kts.append(ktile)

                def q_chain(qb):
                    cs = slice(512 * qb, 512 * qb + 512)
                    acc = ps.tile([128, 512], F32, tag="acc", bufs=1,
                                  name=f"qacc{b}_{qb}")
                    for k in range(KT_C):
                        nc.tensor.matmul(acc[:], wq[k], xt[k][:, cs],
                                         start=(k == 0), stop=(k == KT_C - 1))
                        yield
                    qtile = sb.tile([128, 512], mm_dt, tag="qt", bufs=9,
                                    name=f"qt{b}_{qb}")
                    nc.vector.tensor_copy(qtile[:], acc[:])
                    qt.append(qtile)

                for qb in range(QB):
                    yield from k_chain(qb)
                yield from q_chain(0)
                # V directly in [tok, ch] layout (bf16: full rate at N=128)
                for tt in range(NK):
                    ts_ = slice(128 * tt, 128 * tt + 128)
                    acc = ps.tile([128, 128], F32, tag="acc", bufs=1,
                                  name=f"vacc{b}_{tt}")
                    for k in range(KT_C):
                        nc.tensor.matmul(acc[:], xt[k][:, ts_], wv[k],
                                         start=(k == 0), stop=(k == KT_C - 1))
                        yield
                    nc.vector.tensor_copy(
                        vau[0][:, 65 * tt:65 * tt + 64], acc[:, 0:64])
                    nc.vector.tensor_copy(
                        vau[1][:, 65 * tt:65 * tt + 64], acc[:, 64:128])
                for qb in range(1, QB):
                    yield from q_chain(qb)

            def pull(g, n):
                if g is None:
                    return
                for _ in range(n):
                    if next(g, "done") == "done":
                        return

            gens = [qkv_gen(b) for b in range(B)]
            pull(gens[0], 10 ** 9)  # first batch's qkv emitted up front

            for b in range(B):
                kts, qt, vau = state[b]
                filler = gens[b + 1] if b + 1 < B else None
                # ---- attention, grouped by qb-pairs for chunked AllToAll ----
                grps = QGRP_LAST if b == B - 1 else QGRP
                tok0 = 0
                for gi, grp in enumerate(grps):
                    tokg = 512 * len(grp) // n_cores
                    a2a_in = dr.tile([n_cores, CH, tokg], mm_dt,
                                     tag="a2ain", bufs=2, name=f"a2ain{b}_{gi}")
                    hcps = []
                    for p, qb in enumerate(grp):
                        hav = [ps.tile([65, 512], F32, tag=f"hav{h}", bufs=1,
                                       name=f"hav{b}_{qb}_{h}")
                               for h in range(2)]
                        for t_idx in range(NK):
                            # one tile per key-tile: [h0 | h1] halves, so the
                            # two S^T matmuls alternate PE row groups (paired)
                            sst = ps.tile([128, 1024], F32, tag="sst", bufs=2,
                                          name=f"sst{b}_{qb}_{t_idx}")
                            pt = sb.tile([128, 1024], mm_dt, tag="pt", bufs=6,
                                         name=f"pt{b}_{qb}_{t_idx}")
                            kb, ko = t_idx // 4, t_idx % 4
                            ks = slice(128 * ko, 128 * ko + 128)
                            for h in range(2):
                                hs = slice(64 * h, 64 * h + 64)
                                nc.tensor.matmul(
                                    sst[:, 512 * h:512 * h + 512],
                                    kts[kb][hs, ks], qt[qb][hs, :],
                                    start=True, stop=True)
                            nc.scalar.activation(pt[:], sst[:], EXP,
                                                 scale=scale)
                            for h in range(2):
                                nc.tensor.matmul(
                                    hav[h][:],
                                    vau[h][:, 65 * t_idx:65 * t_idx + 65],
                                    pt[:, 512 * h:512 * h + 512],
                                    start=(t_idx == 0),
                                    stop=(t_idx == NK - 1))
                            # interleave a few of the next batch's qkv matmuls
                            pull(filler, 3)
                        # copy h_aug out of PSUM fast so hav slots recycle
                        hcp = [sb.tile([65, 512], F32, tag=f"hcp{h}", bufs=2,
                                       name=f"hcp{b}_{qb}_{h}")
                               for h in range(2)]
                        for h in range(2):
                            nc.vector.tensor_copy(hcp[h][:], hav[h][:])
                        hcps.append(hcp)
                    for p, qb in enumerate(grp):
                        hcp = hcps[p]
                        ht = sb.tile([128, 512], mm_dt, tag="ht", bufs=2,
                                     name=f"ht{b}_{qb}")
                        for h in range(2):
                            nrr = sb.tile([1, 512], F32, tag=f"nrr{h}",
                                          bufs=2, name=f"nrr{b}_{qb}_{h}")
                            nc.vector.reciprocal(nrr[:], hcp[h][64:65, :])
                            bcs = sb.tile([64, 512], F32, tag=f"bcs{h}",
                                          bufs=2, name=f"bcs{b}_{qb}_{h}")
                            nc.gpsimd.partition_broadcast(bcs[:], nrr[:])
                            nc.vector.tensor_mul(ht[64 * h:64 * h + 64, :],
                                                 hcp[h][0:64, :], bcs[:])
                        # scatter this qb's tokens into the group A2A buffer
                        npr = 512 // tokg
                        for s in range(npr):
                            j = p * npr + s
                            nc.sync.dma_start(
                                a2a_in[j][:, 0:tokg],
                                ht[:, tokg * s:tokg * s + tokg])

                    # ---- AllToAll: head-sharded -> token-sharded ----
                    a2a_out = dr.tile([n_cores, CH, tokg], mm_dt,
                                      tag="a2aout", bufs=2,
                                      name=f"a2aout{b}_{gi}")
                    nc.gpsimd.collective_compute(
                        "AllToAll", mybir.AluOpType.bypass,
                        replica_groups=[list(range(n_cores))],
                        ins=[a2a_in.opt()], outs=[a2a_out.opt()])

                    # ---- output projection for this group's tokens ----
                    if not pw:
                        pwt = sb.tile([128, KT_C * C], mm_dt, tag="pw",
                                      bufs=1, name="pw")
                        nc.sync.dma_start(pwt[:], pwT_d.ap())
                        for k in range(KT_C):
                            pw.append(pwt[:, C * k:C * k + C])
                    pl = []
                    for k in range(KT_C):
                        t = sb.tile([128, tokg], mm_dt, tag=f"pl{k}", bufs=2,
                                    name=f"pl{b}_{gi}_{k}")
                        nc.sync.dma_start(t[:], a2a_out[k])
                        pl.append(t)
                    for oh in range(2):
                        os_ = slice(512 * oh, 512 * oh + 512)
                        acc = ps.tile([128, 512], F32, tag="proj", bufs=1,
                                      name=f"pacc{b}_{gi}_{oh}")
                        for k in range(KT_C):
                            nc.tensor.matmul(
                                acc[0:tokg, :], pl[k][:], pw[k][:, os_],
                                start=(k == 0), stop=False)
                        nc.tensor.matmul(acc[0:tokg, :], onesb[0:1, 0:tokg],
                                         pbb_sb[0:1, os_],
                                         start=False, stop=True)
                        osb = sb.tile([128, 512], F32, tag="osb", bufs=2,
                                      name=f"osb{b}_{gi}_{oh}")
                        nc.vector.tensor_copy(osb[0:tokg, :], acc[0:tokg, :])
                        nc.sync.dma_start(
                            out_d.ap()[b, tok0:tok0 + tokg, os_],
                            osb[0:tokg, :])
                    tok0 += tokg
                pull(filler, 10 ** 9)

    nc.compile()
    return nc


def shard_inputs(x, qkv_w, proj_w, proj_b, n_cores=N_CORES, mm_dt=BF16):
    """Host-side sharding: pre-transpose activations/weights, slice heads."""
    npdt = ml_dtypes.bfloat16 if mm_dt == BF16 else np.float32
    xT = np.ascontiguousarray(
        np.transpose(np.asarray(x), (0, 2, 1))).astype(npdt)
    qkv_w = np.asarray(qkv_w)

    def pack(wT):  # [C, cols] -> [128, (C//128)*cols], k-tiles side by side
        cdim, cols = wT.shape
        return np.ascontiguousarray(
            wT.reshape(cdim // 128, 128, cols).transpose(1, 0, 2)
            .reshape(128, -1)).astype(npdt)

    pwT = pack(np.asarray(proj_w).T)
    pb = np.ascontiguousarray(np.asarray(proj_b)).astype(np.float32)
    nk = x.shape[1] // 128
    vonesc = np.ones((128, nk), dtype=npdt)
    in_maps = []
    for i in range(n_cores):
        cs = slice(CH * i, CH * i + CH)
        in_maps.append({
            "xT": xT,
            "wqT": pack(qkv_w[cs, :].T),
            "wkT": pack(qkv_w[C:][cs, :].T),
            "wvT": pack(qkv_w[2 * C:][cs, :].T),
            "pwT": pwT,
            "onesb": np.ones((1, 128), dtype=npdt),
            "pbb": pb.astype(npdt),
            "vonesc": vonesc,
        })
    return in_maps


def assemble_output(res, N, n_cores=N_CORES):
    QB = N // 512
    QGRP = [list(range(g, min(g + 2, QB))) for g in range(0, QB, 2)]
    if len(QGRP) > 1:
        QGRP_LAST = QGRP[:-1] + [[g] for g in QGRP[-1]]
    else:
        QGRP_LAST = QGRP
    out = np.empty((B, N, C), dtype=np.float32)
    for i in range(n_cores):
        o = res.results[i]["out"]  # [B, TOKB, C]
        for b in range(B):
            grps = QGRP_LAST if b == B - 1 else QGRP
            tok0 = 0
            base = 0
            for grp in grps:
                tokg = 512 * len(grp) // n_cores
                lo = base + tokg * i
                out[b, lo:lo + tokg, :] = o[b, tok0:tok0 + tokg]
                tok0 += tokg
                base += 512 * len(grp)
    return out


_NC_CACHE = {}


def _get_program(N, mm_dt=BF16):
    key = (N, str(mm_dt))
    if key not in _NC_CACHE:
        _NC_CACHE[key] = build_program(N=N, mm_dt=mm_dt)
    return _NC_CACHE[key]


def kernel(x, qkv_w, proj_w, proj_b):
    x = np.asarray(x)
    Bx, N, Cx = x.shape
    assert (Bx, Cx) == (B, C), (Bx, Cx)
    nc = _get_program(N)
    in_maps = shard_inputs(x, qkv_w, proj_w, proj_b)
    res = run_bass_kernel_spmd(nc, in_maps, list(range(N_CORES)))
    return assemble_output(res, N)
